# revision 1
# baseline (speedup 1.0000x reference)
"""Distributed exact k-NN (FAISS IndexFlatL2 semantics) on 8 Trainium2 cores.

Strategy (per the standard distributed exact-kNN recipe):
 - Host: transpose the memory bank to [D, N] layout, shard along N across the
   8 cores, and precompute centered half-squared-norms so the device ranks by
   score = q.m - 0.5*(||m||^2 - D)  (a per-query-constant shift of -d2/2).
 - Device (SPMD, one shard per core): float32r (fast fp32) matmuls compute
   score tiles into PSUM (bias folded in via a K=1 matmul), ScalarE evicts
   tiles to SBUF, and the DVE max/max_index ops extract the top-8 candidates
   (value + index) per 2500-wide slab per query.  One output DMA returns all
   candidates.
 - Host: gathers the per-core candidates, keeps the best 16 per core per
   query, rescores them exactly in float64, and reduces to the global top-k
   (ties broken by lower index, matching jax.lax.top_k).

The per-slab top-8 cut is exact up to score noise: a true global top-5 item
is always within the top 5 of its own slab, and the float32r score noise
(~0.03 in d2 units) cannot push it below rank 8 of a 2500-item slab except
with negligible probability; the 16-per-core host cut has even more margin.
"""

import numpy as np

B, N, D = 256, 500000, 512
NCORES = 8
NLOC = N // NCORES          # 62500 rows per core
FT = 500                    # matmul tile width (one PSUM bank, >=256 for fp32r full rate)
SLAB = 2500                 # DVE max/max_index scan width
NCHUNK = D // 128           # 4 contraction chunks
TOPC = 16                   # candidates kept per core per query on the host

_built = None


def _split_multi_waits(nc):
    """This toolchain's walrus accepts at most one sem-wait/update per
    instruction; Tile attaches full lists.  Split extras into adjacent NoOps
    on the same engine (sequencers execute in order, so this is equivalent)."""
    import concourse.mybir as mybir
    import bass_rust

    counter = [0]
    dma_ops = {"DMACopy", "DMATranspose", "TensorLoad", "TensorSave", "DMAGather"}

    def nop(engine, wait=None, update=None):
        counter[0] += 1
        n = mybir.InstNoOp(name=f"WSPL-{counter[0]}")
        n.engine = engine
        n.sync_info = bass_rust.SyncInfo(
            on_wait=[wait] if wait is not None else [],
            on_update=[update] if update is not None else [],
        )
        return n

    for f in nc.m.functions:
        for bb in f.blocks:
            out = []
            changed = False
            for ins in bb.instructions:
                si = ins.sync_info
                if si is None:
                    out.append(ins)
                    continue
                waits = list(si.on_wait or [])
                updates = list(si.on_update or [])
                if len(waits) <= 1 and len(updates) <= 1:
                    out.append(ins)
                    continue
                changed = True
                for w in waits[:-1]:
                    out.append(nop(ins.engine, wait=w))
                keep_wait = waits[-1:] if waits else []
                if len(updates) > 1:
                    assert ins.opcode not in dma_ops, (
                        f"cannot split updates on DMA inst {ins.name}"
                    )
                    ins.sync_info = bass_rust.SyncInfo(
                        on_wait=keep_wait, on_update=updates[:1]
                    )
                    out.append(ins)
                    for u in updates[1:]:
                        out.append(nop(ins.engine, update=u))
                else:
                    ins.sync_info = bass_rust.SyncInfo(
                        on_wait=keep_wait, on_update=updates
                    )
                    out.append(ins)
            if changed:
                bb.instructions = out


def _build():
    """Build and cache the Bass program (identical for all cores)."""
    global _built
    if _built is not None:
        return _built
    import concourse.bass as bass
    import concourse.tile as tile
    import concourse.mybir as mybir

    nt = NLOC // FT             # matmul tiles per core
    nslab = NLOC // SLAB        # DVE slabs per core
    sub_per_slab = SLAB // FT
    cand = nslab * 8            # candidates per (core, query)
    f32r = mybir.dt.float32r
    f32 = mybir.dt.float32
    u32 = mybir.dt.uint32
    bf16 = mybir.dt.bfloat16

    nc = bass.Bass("TRN2", target_bir_lowering=False, debug=False)
    qT = nc.dram_tensor("qT", [D, B], bf16, kind="ExternalInput")
    memT = nc.dram_tensor("memT", [D, NLOC], bf16, kind="ExternalInput")
    msq = nc.dram_tensor("msq", [nslab, SLAB], f32, kind="ExternalInput")
    out = nc.dram_tensor("out", [128, 4 * cand], f32, kind="ExternalOutput")

    with tile.TileContext(nc) as tc:
        with tc.tile_pool(name="fixed", bufs=1) as fixed_pool, \
             tc.tile_pool(name="mem", bufs=3) as mem_pool, \
             tc.tile_pool(name="msq", bufs=3) as msq_pool, \
             tc.tile_pool(name="msqb", bufs=3) as msqb_pool, \
             tc.tile_pool(name="dist", bufs=3) as dist_pool, \
             tc.tile_pool(name="psum", bufs=6, space="PSUM") as psum_pool:

            qt = fixed_pool.tile([128, NCHUNK, B], bf16)
            nc.sync.dma_start(qt[:], qT.ap().rearrange("(c p) b -> p c b", p=128))
            outsb = fixed_pool.tile([128, 4 * cand], f32)

            memv = memT.ap().rearrange("(c p) n -> p c n", p=128)

            for slab in range(nslab):
                dist = [
                    dist_pool.tile([128, SLAB], f32, tag=f"dist{g}",
                                   name=f"dist{g}_{slab}")
                    for g in (0, 1)
                ]
                mem_t = mem_pool.tile([128, NCHUNK, SLAB], bf16)
                nc.sync.dma_start(
                    mem_t[:], memv[:, :, slab * SLAB:(slab + 1) * SLAB])
                msq_t = msq_pool.tile([1, SLAB], f32)
                nc.gpsimd.dma_start(msq_t[:], msq.ap()[slab:slab + 1, :])
                msqb = msqb_pool.tile([128, SLAB], f32, tag="msqb",
                                      name=f"msqb_{slab}")
                nc.gpsimd.dma_start(msqb[0:1, :], msq_t[:])
                for i in range(7):
                    w = 1 << i
                    nc.gpsimd.dma_start(msqb[w:2 * w, :], msqb[0:w, :])
                for g in (0, 1):
                    pss = [psum_pool.tile([128, FT], f32, tag="ps",
                                          name=f"ps_{slab}_{g}_{s_}")
                           for s_ in range(sub_per_slab)]
                    for c in range(NCHUNK):
                        for sub in range(sub_per_slab):
                            nc.tensor.matmul(
                                pss[sub][:],
                                qt[:, c, g * 128:(g + 1) * 128],
                                mem_t[:, c, sub * FT:(sub + 1) * FT],
                                start=(c == 0), stop=(c == NCHUNK - 1),
                            )
                    for sub in range(sub_per_slab):
                        nc.scalar.copy(dist[g][:, sub * FT:(sub + 1) * FT],
                                       pss[sub][:])
                    nc.vector.tensor_add(dist[g][:], dist[g][:], msqb[:])
                for g in (0, 1):
                    vs = outsb[:, g * cand + slab * 8: g * cand + slab * 8 + 8]
                    nc.vector.max(out=vs, in_=dist[g][:])
                    iv = outsb[:, (2 + g) * cand + slab * 8:
                               (2 + g) * cand + slab * 8 + 8].bitcast(u32)
                    nc.vector.max_index(iv, vs, dist[g][:])

            nc.sync.dma_start(out.ap(), outsb[:])

    _split_multi_waits(nc)
    _built = nc
    return nc


def _run_device(qT_np, memT_np, msqc_np, trace=False):
    """Run the SPMD program on all cores; returns (list of out arrays, exec_ns)."""
    from concourse.bass_utils import run_bass_kernel_spmd

    nc = _build()
    nt = NLOC // FT
    in_maps = []
    for c in range(NCORES):
        in_maps.append({
            "qT": qT_np,
            "memT": np.ascontiguousarray(memT_np[:, c * NLOC:(c + 1) * NLOC]),
            "msq": np.ascontiguousarray(
                msqc_np[c * NLOC:(c + 1) * NLOC].reshape(NLOC // SLAB, SLAB)),
        })
    res = run_bass_kernel_spmd(nc, in_maps, core_ids=list(range(NCORES)),
                               trace=trace)
    outs = [r["out"] for r in res.results]
    return outs, res.exec_time_ns


def kernel(query, memory, k, _trace=False, _return_exec=False):
    k = int(k)
    assert k <= 8
    import ml_dtypes
    query = np.asarray(query, dtype=np.float32)
    memory = np.asarray(memory, dtype=np.float32)
    nslab = NLOC // SLAB
    cand = nslab * 8

    # ---- host-side prep: transpose + centered half squared norms ----
    qT_np = np.ascontiguousarray(query.T).astype(ml_dtypes.bfloat16)   # [D, B]
    memT_np = np.ascontiguousarray(memory.T).astype(ml_dtypes.bfloat16)  # [D, N]
    msq = np.einsum("nd,nd->n", memory, memory)                # [N] fp32
    msqc_np = (-0.5 * (msq - float(D))).astype(np.float32)    # centered bias

    # ---- device: per-core approximate top-8 per slab ----
    outs, exec_ns = _run_device(qT_np, memT_np, msqc_np, trace=_trace)

    # ---- host: decode candidates, exact rescore, global top-k ----
    # per core: vals [B, cand], global idx [B, cand]
    all_vals = np.empty((NCORES, B, cand), dtype=np.float32)
    all_idx = np.empty((NCORES, B, cand), dtype=np.int64)
    slab_base = (np.arange(nslab).repeat(8) * SLAB).astype(np.int64)  # [cand]
    for c in range(NCORES):
        o = outs[c]
        for g in (0, 1):
            vals = o[:, g * cand:(g + 1) * cand]
            lidx = o[:, (2 + g) * cand:(3 + g) * cand].view(np.uint32)
            rows = slice(g * 128, (g + 1) * 128)
            all_vals[c, rows] = vals
            all_idx[c, rows] = c * NLOC + slab_base[None, :] + lidx

    # keep best TOPC per core per query (by approximate score, descending)
    keep = min(TOPC, cand)
    part = np.argpartition(-all_vals, keep - 1, axis=2)[:, :, :keep]
    cvals_idx = np.take_along_axis(all_idx, part, axis=2)      # [NCORES, B, keep]
    cand_idx = np.swapaxes(cvals_idx, 0, 1).reshape(B, NCORES * keep)

    # exact rescore in float64
    q64 = query.astype(np.float64)                             # [B, D]
    qsq = np.sum(q64 * q64, axis=1)                            # [B]
    flat = cand_idx.reshape(-1)
    mrows = memory[flat].astype(np.float64).reshape(B, NCORES * keep, D)
    cross = np.einsum("bd,bcd->bc", q64, mrows)
    msq64 = np.sum(mrows * mrows, axis=2)
    d2 = qsq[:, None] + msq64 - 2.0 * cross                    # [B, NCORES*keep]

    # dedupe is unnecessary (shards are disjoint, slabs are disjoint)
    distances = np.empty((B, k), dtype=np.float32)
    idx = np.empty((B, k), dtype=np.int32)
    for b in range(B):
        order = np.lexsort((cand_idx[b], d2[b]))[:k]
        distances[b] = d2[b][order].astype(np.float32)
        idx[b] = cand_idx[b][order].astype(np.int32)

    if _return_exec:
        return (distances, idx), exec_ns
    return distances, idx



# revision 3
# speedup vs baseline: 3.1933x; 3.1933x over previous
"""Distributed exact k-NN (FAISS IndexFlatL2 semantics) on 8 Trainium2 cores.

Strategy (v2 — fp8 DoubleRow matmul + 1-pass packed top-8 scan):
 - Host: shard the memory bank along N across the 8 cores.  Quantize queries
   and memory to fp8(e4m3, TRN variant) in the DoubleRow-interleaved layout
   [ki=128, chunk=2, ko=2, n].  Feature slot d=511 is sacrificed: q[511]:=1
   and m[511]:=msqc_n = -0.5*(||m_n||^2 - D), folding the L2 bias into the
   matmul (the dropped q_511*m_511 cross term is ~N(0,1) noise, far below the
   candidate-selection margin; candidates are exactly rescored on the host).
 - Device (SPMD, one shard per core): per 2048-item group, 8 DoubleRow fp8
   matmuls (2 k-tiles x 4 psum banks) compute scores q.m+bias into PSUM;
   ScalarE evicts each group as bf16(score+512) written with stride 2 into
   the HIGH u16 halves of a u32 buffer whose LOW halves hold a persistent
   uint16 local-index iota.  The packed u32 value (positive-float bit
   pattern) sorts identically by score, so ONE DVE MAX8 pass per group
   yields the top-8 (value, index) pairs together — no FIND_INDEX8 pass and
   no bias broadcast/add.  One output DMA returns all candidates.
 - Host: decodes (bf16 score, u16 index), keeps the best 16 per core per
   query, rescores them exactly in float64, and reduces to the global top-k
   (ties broken by lower index, matching jax.lax.top_k).

Selection safety: a true global top-5 item must stay in the top-8 of its
2048-item group under the score noise (fp8 quantization ~1.1, sacrificed
dim ~1.0, bf16 packing ~0.4, all std, in q.m units).  The gap between a
global top-5 score (>=4.3 sigma of the 22.6-sigma score spread) and a
group's 8th-best (~2.9 sigma) is >25x the combined noise std.
"""

import numpy as np

B, N, D = 256, 500000, 512
NCORES = 8
NLOC = N // NCORES          # 62500 rows per core
TILE = 512                  # one PSUM bank of fp32
TPG = 4                     # psum banks (tiles) per group
GW = TILE * TPG             # 2048-item group = one MAX8 scan
NGRP = 31                   # groups per core
NLOC_PAD = GW * NGRP        # 63488 (>= NLOC, padded with dummies)
NPK = 4                     # rotating packed-score buffers
TOPC = 16                   # candidates kept per core per query on the host
SHIFT = 512.0               # makes all packed scores positive

_built = None


def _split_multi_waits(nc):
    """This toolchain's walrus accepts at most one sem-wait/update per
    instruction; Tile attaches full lists.  Split extras into adjacent NoOps
    on the same engine (sequencers execute in order, so this is equivalent)."""
    import concourse.mybir as mybir
    import bass_rust

    counter = [0]
    dma_ops = {"DMACopy", "DMATranspose", "TensorLoad", "TensorSave", "DMAGather"}

    def nop(engine, wait=None, update=None):
        counter[0] += 1
        n = mybir.InstNoOp(name=f"WSPL-{counter[0]}")
        n.engine = engine
        n.sync_info = bass_rust.SyncInfo(
            on_wait=[wait] if wait is not None else [],
            on_update=[update] if update is not None else [],
        )
        return n

    for f in nc.m.functions:
        for bb in f.blocks:
            out = []
            changed = False
            for ins in bb.instructions:
                si = ins.sync_info
                if si is None:
                    out.append(ins)
                    continue
                waits = list(si.on_wait or [])
                updates = list(si.on_update or [])
                if len(waits) <= 1 and len(updates) <= 1:
                    out.append(ins)
                    continue
                changed = True
                for w in waits[:-1]:
                    out.append(nop(ins.engine, wait=w))
                keep_wait = waits[-1:] if waits else []
                if len(updates) > 1:
                    assert ins.opcode not in dma_ops, (
                        f"cannot split updates on DMA inst {ins.name}"
                    )
                    ins.sync_info = bass_rust.SyncInfo(
                        on_wait=keep_wait, on_update=updates[:1]
                    )
                    out.append(ins)
                    for u in updates[1:]:
                        out.append(nop(ins.engine, update=u))
                else:
                    ins.sync_info = bass_rust.SyncInfo(
                        on_wait=keep_wait, on_update=updates
                    )
                    out.append(ins)
            if changed:
                bb.instructions = out


def _build(ngrp=NGRP):
    """Build the Bass program (identical for all cores)."""
    global _built
    if ngrp == NGRP and _built is not None:
        return _built
    import concourse.bass as bass
    import concourse.tile as tile
    import concourse.mybir as mybir

    f32 = mybir.dt.float32
    bf16 = mybir.dt.bfloat16
    fp8 = mybir.dt.float8e4
    nloc_pad = GW * ngrp
    ncand = ngrp * 8
    DR = mybir.MatmulPerfMode.DoubleRow
    Ident = mybir.ActivationFunctionType.Identity

    nc = bass.Bass("TRN2", target_bir_lowering=False, debug=False)
    shift_t = nc.alloc_sbuf_tensor("const-shift", [128, 1], f32)
    nc.gpsimd.memset(shift_t.ap(), SHIFT)
    nc.const_aps.aps[(f32, SHIFT)] = shift_t.ap()
    nc.all_engine_barrier()
    qT = nc.dram_tensor("qT", [128, 2, 2, B], fp8, kind="ExternalInput")
    memT = nc.dram_tensor("memT", [128, 2, 2, nloc_pad], fp8, kind="ExternalInput")
    iota = nc.dram_tensor("iota", [128, GW, 2], bf16, kind="ExternalInput")
    out = nc.dram_tensor("out", [128, 2 * ncand], f32, kind="ExternalOutput")

    with tile.TileContext(nc) as tc:
        with tc.tile_pool(name="fixed", bufs=1) as fixed_pool, \
             tc.tile_pool(name="mem", bufs=3) as mem_pool, \
             tc.tile_pool(name="psum", bufs=2, space="PSUM") as psum_pool:

            qt = fixed_pool.tile([128, 2, 2, B], fp8)
            nc.sync.dma_start(qt[:], qT.ap())
            pk = []
            for i in range(NPK):
                t = fixed_pool.tile([128, GW, 2], bf16, name=f"pk{i}")
                nc.gpsimd.dma_start(t[:], iota.ap())
                pk.append(t)
            outsb = fixed_pool.tile([128, 2 * ncand], f32)

            memv = memT.ap()
            for s in range(ngrp):
                mt = mem_pool.tile([128, 2, 2, GW], fp8)
                nc.sync.dma_start(mt[:], memv[:, :, :, s * GW:(s + 1) * GW])
                for g in (0, 1):
                    ps = psum_pool.tile([128, GW], f32, tag="ps",
                                        name=f"ps_{s}_{g}")
                    for c in (0, 1):
                        for t in range(TPG):
                            nc.tensor.matmul(
                                ps[:, t * TILE:(t + 1) * TILE],
                                qt[:, c, :, g * 128:(g + 1) * 128],
                                mt[:, c, :, t * TILE:(t + 1) * TILE],
                                start=(c == 0), stop=(c == 1),
                                perf_mode=DR,
                            )
                    pkt = pk[(2 * s + g) % NPK]
                    # bf16(score+512) into the high u16 of each packed u32
                    nc.scalar.activation(pkt[:, :, 1:2], ps[:], Ident,
                                         bias=SHIFT)
                    # one MAX8 pass over the packed u32s = top-8 (val, idx)
                    k8 = (g * ngrp + s) * 8
                    nc.vector.max(out=outsb[:, k8:k8 + 8],
                                  in_=pkt.bitcast(f32))

            nc.sync.dma_start(out.ap(), outsb[:])

    _split_multi_waits(nc)
    if ngrp == NGRP:
        _built = nc
    return nc


def _host_prep(query, memory):
    """Quantize + lay out inputs for the device program."""
    import ml_dtypes
    fp8 = ml_dtypes.float8_e4m3

    msq = np.einsum("nd,nd->n", memory, memory)                # [N] fp32
    msqc = (-0.5 * (msq - float(D))).astype(np.float32)        # centered bias

    qmod = np.array(query, dtype=np.float32)
    qmod[:, D - 1] = 1.0
    q8 = np.clip(qmod, -240, 240).astype(fp8)                  # [B, 512]
    # [ki, c, ko, b] with d = c*256 + ko*128 + ki
    qT8 = np.ascontiguousarray(
        q8.T.reshape(2, 2, 128, B).transpose(2, 0, 1, 3))

    mem8_cores = []
    for cidx in range(NCORES):
        mm = np.array(memory[cidx * NLOC:(cidx + 1) * NLOC], dtype=np.float32)
        mm[:, D - 1] = msqc[cidx * NLOC:(cidx + 1) * NLOC]
        m8 = np.clip(mm, -240, 240).astype(fp8)                # [NLOC, 512]
        pad = np.zeros((NLOC_PAD - NLOC, D), dtype=fp8)
        pad[:, D - 1] = fp8(-240.0)                            # dummies lose
        m8 = np.concatenate([m8, pad], axis=0)                 # [NLOC_PAD, 512]
        mem8_cores.append(np.ascontiguousarray(
            m8.T.reshape(2, 2, 128, NLOC_PAD).transpose(2, 0, 1, 3)))

    iota = np.zeros((128, GW, 2), dtype=np.uint16)
    iota[:, :, 0] = np.arange(GW, dtype=np.uint16)[None, :]
    iota = iota.view(ml_dtypes.bfloat16)
    return qT8, mem8_cores, iota


def _run_device(qT8, mem8_cores, iota, trace=False):
    from concourse.bass_utils import run_bass_kernel_spmd

    nc = _build()
    in_maps = [{"qT": qT8, "memT": mem8_cores[c], "iota": iota}
               for c in range(NCORES)]
    res = run_bass_kernel_spmd(nc, in_maps, core_ids=list(range(NCORES)),
                               trace=trace)
    outs = [r["out"] for r in res.results]
    return outs, res.exec_time_ns


def kernel(query, memory, k, _trace=False, _return_exec=False):
    import ml_dtypes
    k = int(k)
    assert k <= 8
    query = np.asarray(query, dtype=np.float32)
    memory = np.asarray(memory, dtype=np.float32)
    ncand = NGRP * 8

    qT8, mem8_cores, iota = _host_prep(query, memory)
    outs, exec_ns = _run_device(qT8, mem8_cores, iota, trace=_trace)

    # ---- host: decode candidates, exact rescore, global top-k ----
    grp_base = (np.arange(NGRP, dtype=np.int64) * GW)[None, :, None]
    all_vals = np.empty((NCORES, B, ncand), dtype=np.float32)
    all_idx = np.empty((NCORES, B, ncand), dtype=np.int64)
    for c in range(NCORES):
        u = outs[c].view(np.uint32)                            # [128, 2*ncand]
        for g in (0, 1):
            blk = u[:, g * ncand:(g + 1) * ncand].reshape(128, NGRP, 8)
            lidx = (blk & 0xFFFF).astype(np.int64) + grp_base  # [128,NGRP,8]
            vals = ((blk >> 16).astype(np.uint16)
                    .view(ml_dtypes.bfloat16).astype(np.float32))
            valid = lidx < NLOC
            vals = np.where(valid, vals, -np.inf)
            rows = slice(g * 128, (g + 1) * 128)
            all_vals[c, rows] = vals.reshape(128, ncand)
            all_idx[c, rows] = np.where(
                valid, c * NLOC + lidx, 0).reshape(128, ncand)

    # keep best TOPC per core per query (by approximate score, descending)
    keep = min(TOPC, ncand)
    part = np.argpartition(-all_vals, keep - 1, axis=2)[:, :, :keep]
    cvals_idx = np.take_along_axis(all_idx, part, axis=2)      # [NCORES,B,keep]
    cand_idx = np.swapaxes(cvals_idx, 0, 1).reshape(B, NCORES * keep)

    # exact rescore in float64
    q64 = query.astype(np.float64)                             # [B, D]
    qsq = np.sum(q64 * q64, axis=1)                            # [B]
    flat = cand_idx.reshape(-1)
    mrows = memory[flat].astype(np.float64).reshape(B, NCORES * keep, D)
    cross = np.einsum("bd,bcd->bc", q64, mrows)
    msq64 = np.sum(mrows * mrows, axis=2)
    d2 = qsq[:, None] + msq64 - 2.0 * cross                    # [B, NCORES*keep]

    distances = np.empty((B, k), dtype=np.float32)
    idx = np.empty((B, k), dtype=np.int32)
    for b in range(B):
        order = np.lexsort((cand_idx[b], d2[b]))[:k]
        distances[b] = d2[b][order].astype(np.float32)
        idx[b] = cand_idx[b][order].astype(np.int32)

    if _return_exec:
        return (distances, idx), exec_ns
    return (distances, idx)


# revision 10
# speedup vs baseline: 3.7681x; 1.1800x over previous
"""Distributed exact k-NN (FAISS IndexFlatL2 semantics) on 8 Trainium2 cores.

Strategy (v4 — fp8 DoubleRow matmul + bf16 max fold-tree, no top-k scan):
 - Host: shard the memory bank along N across the 8 cores.  Quantize queries
   and memory to fp8(e4m3, TRN variant) in the DoubleRow-interleaved layout
   [ki=128, chunk=2, ko=2, n].  Feature slot d=511 is sacrificed: q[511]:=1
   and m[511]:=msqc_n = -0.5*(||m_n||^2 - D), folding the L2 bias into the
   matmul (the dropped q_511*m_511 cross term is ~N(0,1) noise, far below the
   candidate-selection margin; candidates are exactly rescored on the host).
 - Device (SPMD, one shard per core): per 2048-item group, 8 DoubleRow fp8
   matmuls (2 k-tiles x 4 psum banks) compute scores q.m+bias into PSUM.
   ScalarE evicts banks 0-2 to bf16; VectorE pair-maxes bank 3 straight out
   of PSUM (dual-port read), then three bf16 tensor_max folds (2x DVE rate)
   reduce the group to 256 segment maxima, where segment i covers the 8
   items == i (mod 256).  No MAX8/FIND_INDEX8 scans, no index packing —
   positions are implicit in the fold-tree wiring.  Segment maxima stream
   out per superslab (31 x 131 KB DMAs, overlapped).
 - Host: per (query, core) picks the top-10 segments by bf16 segment-max,
   expands their 8 members each into 80 candidates, rescores all of them
   exactly in float64, and reduces to the global top-k (ties broken by
   lower index, matching jax.lax.top_k).

Selection safety: a true global top-5 item's segment-max is >= its score,
so only segments containing a strictly better item (at most 4, plus ~2 for
the +-1 bf16 rounding of segment maxima against a >25-sigma score margin)
can outrank its segment — top-10 per core is ample.  Exactness comes from
the host-side float64 rescore of every candidate member.
"""

import numpy as np

B, N, D = 256, 500000, 512
NCORES = 8
NLOC = N // NCORES          # 62500 rows per core
TILE = 512                  # one PSUM bank of fp32
TPG = 4                     # psum banks (tiles) per group
GW = TILE * TPG             # 2048-item group
NGRP = 31                   # groups per core
NLOC_PAD = GW * NGRP        # 63488 (>= NLOC, padded with dummies)
SEG = 256                   # segment maxima per group (8 members each)
ACT_W = 1792                # ScalarE evicts [0:1792), VectorE [1792:2048)
NSEGC = 10                  # segments kept per core per query on the host

_built = None


def _split_multi_waits(nc):
    """This toolchain's walrus accepts at most one sem-wait/update per
    instruction; Tile attaches full lists.  Split extras into adjacent NoOps
    on the same engine (sequencers execute in order, so this is equivalent)."""
    import concourse.mybir as mybir
    import bass_rust

    counter = [0]
    dma_ops = {"DMACopy", "DMATranspose", "TensorLoad", "TensorSave", "DMAGather"}

    def nop(engine, wait=None, update=None):
        counter[0] += 1
        n = mybir.InstNoOp(name=f"WSPL-{counter[0]}")
        n.engine = engine
        n.sync_info = bass_rust.SyncInfo(
            on_wait=[wait] if wait is not None else [],
            on_update=[update] if update is not None else [],
        )
        return n

    for f in nc.m.functions:
        for bb in f.blocks:
            out = []
            changed = False
            for ins in bb.instructions:
                si = ins.sync_info
                if si is None:
                    out.append(ins)
                    continue
                waits = list(si.on_wait or [])
                updates = list(si.on_update or [])
                if len(waits) <= 1 and len(updates) <= 1:
                    out.append(ins)
                    continue
                changed = True
                for w in waits[:-1]:
                    out.append(nop(ins.engine, wait=w))
                keep_wait = waits[-1:] if waits else []
                if len(updates) > 1:
                    assert ins.opcode not in dma_ops, (
                        f"cannot split updates on DMA inst {ins.name}"
                    )
                    ins.sync_info = bass_rust.SyncInfo(
                        on_wait=keep_wait, on_update=updates[:1]
                    )
                    out.append(ins)
                    for u in updates[1:]:
                        out.append(nop(ins.engine, update=u))
                else:
                    ins.sync_info = bass_rust.SyncInfo(
                        on_wait=keep_wait, on_update=updates
                    )
                    out.append(ins)
            if changed:
                bb.instructions = out


def _build(ngrp=NGRP):
    """Build the Bass program (identical for all cores)."""
    global _built
    if ngrp == NGRP and _built is not None:
        return _built
    import concourse.bass as bass
    import concourse.tile as tile
    import concourse.mybir as mybir

    f32 = mybir.dt.float32
    bf16 = mybir.dt.bfloat16
    fp8 = mybir.dt.float8e4
    nloc_pad = GW * ngrp
    DR = mybir.MatmulPerfMode.DoubleRow

    nc = bass.Bass("TRN2", target_bir_lowering=False, debug=False)
    qT = nc.dram_tensor("qT", [128, 2, 2, B], fp8, kind="ExternalInput")
    memT = nc.dram_tensor("memT", [128, 2, 2, nloc_pad], fp8, kind="ExternalInput")
    out = nc.dram_tensor("out", [128, ngrp, 2, SEG], bf16, kind="ExternalOutput")

    with tile.TileContext(nc) as tc:
        with tc.tile_pool(name="fixed", bufs=1) as fixed_pool, \
             tc.tile_pool(name="mem", bufs=3) as mem_pool, \
             tc.tile_pool(name="aev", bufs=3) as a_pool, \
             tc.tile_pool(name="t1", bufs=3) as t1_pool, \
             tc.tile_pool(name="ct", bufs=3) as c_pool, \
             tc.tile_pool(name="outp", bufs=3) as out_pool, \
             tc.tile_pool(name="psum", bufs=2, space="PSUM") as psum_pool:

            qt = fixed_pool.tile([128, 2, 2, B], fp8)
            nc.sync.dma_start(qt[:], qT.ap())

            memv = memT.ap()
            for s in range(ngrp):
                mt = mem_pool.tile([128, 2, 2, GW], fp8)
                nc.sync.dma_start(mt[:], memv[:, :, :, s * GW:(s + 1) * GW])
                ot = out_pool.tile([128, 2, SEG], bf16, tag="ot",
                                   name=f"ot_{s}")
                for g in (0, 1):
                    ps = psum_pool.tile([128, GW], f32, tag="ps",
                                        name=f"ps_{s}_{g}")
                    for c in (0, 1):
                        for t in range(TPG):
                            nc.tensor.matmul(
                                ps[:, t * TILE:(t + 1) * TILE],
                                qt[:, c, :, g * 128:(g + 1) * 128],
                                mt[:, c, :, t * TILE:(t + 1) * TILE],
                                start=(c == 0), stop=(c == 1),
                                perf_mode=DR,
                            )
                    # fold tree: 2048 scores -> 256 segment maxima, where
                    # segment i covers items == i (mod 256) of the group
                    at = a_pool.tile([128, GW], bf16, tag="at",
                                     name=f"at_{s}_{g}")
                    nc.scalar.copy(at[:, 0:ACT_W], ps[:, 0:ACT_W])
                    nc.vector.tensor_copy(at[:, ACT_W:GW], ps[:, ACT_W:GW])
                    t1 = t1_pool.tile([128, 1024], bf16, tag="t1",
                                      name=f"t1_{s}_{g}")
                    nc.vector.tensor_max(t1[:], at[:, 0:1024], at[:, 1024:GW])
                    ct = c_pool.tile([128, 512], bf16, tag="ct",
                                     name=f"ct_{s}_{g}")
                    nc.vector.tensor_max(ct[:], t1[:, 0:512], t1[:, 512:1024])
                    nc.vector.tensor_max(ot[:, g, :],
                                         ct[:, 0:256], ct[:, 256:512])
                nc.sync.dma_start(out.ap()[:, s], ot[:])

    _split_multi_waits(nc)
    if ngrp == NGRP:
        _built = nc
    return nc


def _host_prep(query, memory):
    """Quantize + lay out inputs for the device program."""
    import ml_dtypes
    fp8 = ml_dtypes.float8_e4m3

    msq = np.einsum("nd,nd->n", memory, memory)                # [N] fp32
    msqc = (-0.5 * (msq - float(D))).astype(np.float32)        # centered bias

    qmod = np.array(query, dtype=np.float32)
    qmod[:, D - 1] = 1.0
    q8 = np.clip(qmod, -240, 240).astype(fp8)                  # [B, 512]
    # [ki, c, ko, b] with d = c*256 + ko*128 + ki
    qT8 = np.ascontiguousarray(
        q8.T.reshape(2, 2, 128, B).transpose(2, 0, 1, 3))

    mem8_cores = []
    for cidx in range(NCORES):
        mm = np.array(memory[cidx * NLOC:(cidx + 1) * NLOC], dtype=np.float32)
        mm[:, D - 1] = msqc[cidx * NLOC:(cidx + 1) * NLOC]
        m8 = np.clip(mm, -240, 240).astype(fp8)                # [NLOC, 512]
        pad = np.zeros((NLOC_PAD - NLOC, D), dtype=fp8)
        pad[:, D - 1] = fp8(-240.0)                            # dummies lose
        m8 = np.concatenate([m8, pad], axis=0)                 # [NLOC_PAD, 512]
        mem8_cores.append(np.ascontiguousarray(
            m8.T.reshape(2, 2, 128, NLOC_PAD).transpose(2, 0, 1, 3)))
    return qT8, mem8_cores


def _run_device(qT8, mem8_cores, trace=False):
    from concourse.bass_utils import run_bass_kernel_spmd

    nc = _build()
    in_maps = [{"qT": qT8, "memT": mem8_cores[c]} for c in range(NCORES)]
    res = run_bass_kernel_spmd(nc, in_maps, core_ids=list(range(NCORES)),
                               trace=trace)
    outs = [r["out"] for r in res.results]
    return outs, res.exec_time_ns


def kernel(query, memory, k, _trace=False, _return_exec=False):
    import ml_dtypes
    k = int(k)
    assert k <= 8
    query = np.asarray(query, dtype=np.float32)
    memory = np.asarray(memory, dtype=np.float32)

    qT8, mem8_cores = _host_prep(query, memory)
    outs, exec_ns = _run_device(qT8, mem8_cores, trace=_trace)

    # ---- host: top segments per core, expand members, exact rescore ----
    nseg = NGRP * SEG
    segv = np.empty((NCORES, B, nseg), dtype=np.float32)
    for c in range(NCORES):
        v = outs[c].astype(np.float32)             # [128, NGRP, 2, SEG]
        for g in (0, 1):
            segv[c, g * 128:(g + 1) * 128] = v[:, :, g, :].reshape(128, nseg)

    top = np.argpartition(-segv, NSEGC - 1, axis=2)[:, :, :NSEGC]
    # segment t -> group s = t // SEG, slot i = t % SEG;
    # members: local idx = s*GW + i + 256*np.arange(8)
    s_id = top // SEG
    i_id = top % SEG
    base = s_id * GW + i_id                        # [NCORES, B, NSEGC]
    lidx = base[..., None] + (SEG * np.arange(8))[None, None, None, :]
    lidx = lidx.reshape(NCORES, B, NSEGC * 8)      # [NCORES, B, 80]
    valid = lidx < NLOC
    gidx = np.arange(NCORES)[:, None, None] * NLOC + np.where(valid, lidx, 0)

    cand_idx = np.swapaxes(gidx, 0, 1).reshape(B, NCORES * NSEGC * 8)
    cand_ok = np.swapaxes(valid, 0, 1).reshape(B, NCORES * NSEGC * 8)

    # exact rescore in float64
    q64 = query.astype(np.float64)                 # [B, D]
    qsq = np.sum(q64 * q64, axis=1)                # [B]
    flat = cand_idx.reshape(-1)
    nc_ = cand_idx.shape[1]
    mrows = memory[flat].astype(np.float64).reshape(B, nc_, D)
    cross = np.einsum("bd,bcd->bc", q64, mrows)
    msq64 = np.sum(mrows * mrows, axis=2)
    d2 = qsq[:, None] + msq64 - 2.0 * cross        # [B, nc_]
    d2 = np.where(cand_ok, d2, np.inf)

    distances = np.empty((B, k), dtype=np.float32)
    idx = np.empty((B, k), dtype=np.int32)
    for b in range(B):
        order = np.lexsort((cand_idx[b], d2[b]))[:k]
        distances[b] = d2[b][order].astype(np.float32)
        idx[b] = cand_idx[b][order].astype(np.int32)

    if _return_exec:
        return (distances, idx), exec_ns
    return (distances, idx)
